# revision 9
# baseline (speedup 1.0000x reference)
"""Multi-head causal attention (B=2, S=2048, D=1024, H=16) on 8 trn2 NeuronCores.

Sharding: core c handles batch b = c//4 and head group g = c%4 (heads 4g..4g+3).
Each core computes:
  qkv projection for its 4 heads        [2048,1024] @ [1024,3*256]
  causal attention for its 4 heads      (scoresT layout, softmax w/o max-sub)
  partial output projection             ctx_c @ w_out[rows] -> [2048,1024]
Host sums the 4 partial outputs per batch.

All matmuls use float32r (11-bit mantissa, full PE rate). Inputs consumed by
matmuls are pre-rounded to f32r on the host; on-chip PSUM evictions round via
the ACT copy that writes f32r SBUF tiles.
"""

import sys
from contextlib import ExitStack

for _p in ("/opt/trn_rl_repo",):
    if _p not in sys.path:
        sys.path.insert(0, _p)

import numpy as np

import concourse.bass as bass  # noqa: F401
import concourse.tile as tile
from concourse import bacc, bass_utils, mybir

B, S, D, H, HD = 2, 2048, 1024, 16, 64
P = 128
NCORES = 8
NT = S // P          # 16 token tiles
KD = D // P          # 8 contraction tiles over D
NB = S // 512        # 4 query blocks of 512
HPC = 4              # heads per core
WCOLS = HPC * HD     # 256 weight columns per core per q/k/v

F32 = mybir.dt.float32
F32R = mybir.dt.float32r
BF16 = mybir.dt.bfloat16
EXP = mybir.ActivationFunctionType.Exp

import os as _os
_DT_NAMES = {"f32r": F32R, "bf16": BF16}
DT_PROJ = _DT_NAMES[_os.environ.get("KERNEL_DT_PROJ", "bf16")]
DT_ATTN = _DT_NAMES[_os.environ.get("KERNEL_DT_ATTN", "bf16")]

def round_f32r(x: np.ndarray) -> np.ndarray:
    """Round fp32 to nearest f32r (11 mantissa bits kept), matching PE HW."""
    b = np.ascontiguousarray(x, dtype=np.float32).view(np.uint32)
    r = (b + np.uint32(0x7FF) + ((b >> np.uint32(12)) & np.uint32(1))) & np.uint32(
        0xFFFFF000
    )
    return r.view(np.float32)


def prep(x: np.ndarray, dt) -> np.ndarray:
    """Convert host fp32 array to the numpy form matching DRAM dtype dt."""
    if dt is F32R:
        return round_f32r(x)
    import ml_dtypes

    return np.ascontiguousarray(x, np.float32).astype(ml_dtypes.bfloat16)


def _emit(tc: tile.TileContext, aps: dict):
    nc = tc.nc
    xT, wq, wk, wv, wo, tri, out = (
        aps["xT"], aps["wq"], aps["wk"], aps["wv"], aps["wo"],
        aps["tri"], aps["out"],
    )

    with ExitStack() as top:
        # --- persistent pools -------------------------------------------------
        qk_pool = top.enter_context(tc.tile_pool(name="qk", bufs=4))
        v_pool = top.enter_context(tc.tile_pool(name="v1", bufs=NT))
        ctx_pool = top.enter_context(tc.tile_pool(name="ctxT", bufs=2))
        wo_pool = top.enter_context(tc.tile_pool(name="wo", bufs=2))
        const_pool = top.enter_context(tc.tile_pool(name="const", bufs=1))
        small_pool = top.enter_context(tc.tile_pool(name="small", bufs=2))
        out_pool = top.enter_context(tc.tile_pool(name="outsb", bufs=3))
        ps = top.enter_context(tc.tile_pool(name="ps", bufs=3, space="PSUM"))

        # persistent SBUF tiles
        qT = [qk_pool.tile([P, S], DT_ATTN, tag="qk", name=f"qT{i}") for i in range(2)]
        kT = [qk_pool.tile([P, S], DT_ATTN, tag="qk", name=f"kT{i}") for i in range(2)]
        v1 = [v_pool.tile([P, HPC * (HD + 1)], DT_ATTN, tag="v1", name=f"v1_{i}") for i in range(NT)]
        ctxT = [ctx_pool.tile([P, S], DT_PROJ, tag="ctxT", name=f"ctxT{i}") for i in range(2)]
        wo_sb = [wo_pool.tile([P, D], DT_PROJ, tag="wo", name=f"wo{i}") for i in range(2)]
        tri_sb = const_pool.tile([P, P], DT_ATTN, tag="tri")
        ones4 = const_pool.tile([P, HPC], F32, tag="ones4")
        nc.vector.memset(ones4[:], 1.0)

        nc.sync.dma_start(tri_sb[:], tri[:])
        for i in range(2):
            nc.sync.dma_start(wo_sb[i][:], wo[i * P : (i + 1) * P, :])

        # ===== Phase 1: qkv projection =======================================
        with ExitStack() as ph1:
            x_pool = ph1.enter_context(tc.tile_pool(name="xc", bufs=32))
            w_pool = ph1.enter_context(tc.tile_pool(name="w", bufs=3 * KD))

            wq_sb = [w_pool.tile([P, WCOLS], DT_PROJ, tag="w", name=f"wq{i}") for i in range(KD)]
            wk_sb = [w_pool.tile([P, WCOLS], DT_PROJ, tag="w", name=f"wk{i}") for i in range(KD)]
            wv_sb = [w_pool.tile([P, WCOLS], DT_PROJ, tag="w", name=f"wv{i}") for i in range(KD)]
            xc = {}

            # DMA emission order: interleave so compute can start early.
            for kt in range(KD):
                nc.sync.dma_start(wq_sb[kt][:], wq[kt * P : (kt + 1) * P, :])
            for kt in range(KD):
                xc[(kt, 0)] = x_pool.tile([P, 512], DT_PROJ, tag="xc", name=f"xc{kt}_0")
                nc.sync.dma_start(xc[(kt, 0)][:], xT[kt * P : (kt + 1) * P, 0:512])
            for kt in range(KD):
                nc.sync.dma_start(wk_sb[kt][:], wk[kt * P : (kt + 1) * P, :])
            for kt in range(KD):
                xc[(kt, 1)] = x_pool.tile([P, 512], DT_PROJ, tag="xc", name=f"xc{kt}_1")
                nc.sync.dma_start(xc[(kt, 1)][:], xT[kt * P : (kt + 1) * P, 512:1024])
            for kt in range(KD):
                nc.sync.dma_start(wv_sb[kt][:], wv[kt * P : (kt + 1) * P, :])
            for nb in (2, 3):
                for kt in range(KD):
                    xc[(kt, nb)] = x_pool.tile([P, 512], DT_PROJ, tag="xc", name=f"xc{kt}_{nb}")
                    nc.sync.dma_start(
                        xc[(kt, nb)][:],
                        xT[kt * P : (kt + 1) * P, nb * 512 : (nb + 1) * 512],
                    )

            for nb in range(NB):
                cols = slice(nb * 512, (nb + 1) * 512)
                # qT: scale 1/sqrt(HD) folded in here
                for p in range(2):
                    psq = ps.tile([P, 512], F32, tag="pss")
                    for kt in range(KD):
                        nc.tensor.matmul(
                            psq[:],
                            wq_sb[kt][:, p * P : (p + 1) * P],
                            xc[(kt, nb)][:],
                            start=(kt == 0),
                            stop=(kt == KD - 1),
                        )
                    nc.scalar.mul(qT[p][:, cols], psq[:], 1.0 / np.sqrt(HD))
                for p in range(2):
                    psk = ps.tile([P, 512], F32, tag="pss")
                    for kt in range(KD):
                        nc.tensor.matmul(
                            psk[:],
                            wk_sb[kt][:, p * P : (p + 1) * P],
                            xc[(kt, nb)][:],
                            start=(kt == 0),
                            stop=(kt == KD - 1),
                        )
                    nc.scalar.copy(kT[p][:, cols], psk[:])
                for tloc in range(4):
                    tt = nb * 4 + tloc
                    psv = ps.tile([P, 512], F32, tag="pss")
                    for kt in range(KD):
                        nc.tensor.matmul(
                            psv[:, 0:WCOLS],
                            xc[(kt, nb)][:, tloc * P : (tloc + 1) * P],
                            wv_sb[kt][:],
                            start=(kt == 0),
                            stop=(kt == KD - 1),
                        )
                    v1_view = v1[tt][:].rearrange("p (a c) -> p a c", c=HD + 1)
                    nc.scalar.copy(
                        v1_view[:, :, 0:HD],
                        psv[:, 0:WCOLS].rearrange("p (a c) -> p a c", c=HD),
                    )
                    nc.scalar.copy(
                        v1_view[:, :, HD : HD + 1],
                        ones4[:].rearrange("p (a c) -> p a c", c=1),
                    )

        # ===== Phase 2: attention ============================================
        with ExitStack() as ph2:
            exp_pool = ph2.enter_context(tc.tile_pool(name="expT", bufs=6))
            ctxps_pool = ph2.enter_context(
                tc.tile_pool(name="ctxps", bufs=5, space="PSUM")
            )

            for h in range(HPC):
                p, off = h // 2, 64 * (h % 2)
                rec4 = small_pool.tile([P, 512], F32, tag="rec4", name=f"rec4_{h}")
                rec4i = small_pool.tile([P, 512], F32, tag="rec4i", name=f"rec4i_{h}")
                nc.vector.memset(rec4[:], 1.0)
                ctxps_list = []
                for qb in range(NB):
                    njt = 4 * qb + 4
                    q0 = qb * 512
                    exps = []
                    for jt in range(njt):
                        m = jt - 4 * qb
                        lo = P * m if m > 0 else 0
                        pss = ps.tile([P, 512], F32, tag="pss")
                        nc.tensor.matmul(
                            pss[:, lo:512],
                            kT[p][off : off + 64, jt * P : (jt + 1) * P],
                            qT[p][off : off + 64, q0 + lo : q0 + 512],
                            start=True,
                            stop=True,
                        )
                        et = exp_pool.tile([P, 512], DT_ATTN, tag="expT")
                        nc.scalar.activation(et[:, lo:512], pss[:, lo:512], EXP)
                        if m >= 0:  # diagonal 128-block: triangle mask multiply
                            nc.vector.tensor_mul(
                                et[:, lo : lo + P], et[:, lo : lo + P], tri_sb[:]
                            )
                        exps.append((et, lo))

                    ctxps = ctxps_pool.tile([65, 512], F32, tag="ctxps")
                    ctxps_list.append(ctxps)
                    for jt in range(njt):
                        et, lo = exps[jt]
                        nc.tensor.matmul(
                            ctxps[:, lo:512],
                            v1[jt][:, h * 65 : (h + 1) * 65],
                            et[:, lo:512],
                            start=(jt == 0),
                            stop=(jt == njt - 1),
                        )
                    nc.vector.tensor_copy(rec4[32 * qb : 32 * qb + 1, :], ctxps[64:65, :])
                nc.vector.reciprocal(rec4i[:], rec4[:])
                for qb in range(NB):
                    rec_s = small_pool.tile([1, 512], F32, tag="rec_s")
                    nc.vector.tensor_copy(rec_s[:], rec4i[32 * qb : 32 * qb + 1, :])
                    recb = small_pool.tile([64, 512], F32, tag="recb")
                    nc.gpsimd.partition_broadcast(recb[:], rec_s[:], channels=64)
                    nc.vector.tensor_mul(
                        ctxT[p][off : off + 64, qb * 512 : (qb + 1) * 512],
                        ctxps_list[qb][0:64, :],
                        recb[:],
                    )

        # ===== Phase 3: output projection (partial) ==========================
        for tt in range(NT):
            for ob in range(2):
                pso = ps.tile([P, 512], F32, tag="pss")
                for kt2 in range(2):
                    nc.tensor.matmul(
                        pso[:],
                        ctxT[kt2][:, tt * P : (tt + 1) * P],
                        wo_sb[kt2][:, ob * 512 : (ob + 1) * 512],
                        start=(kt2 == 0),
                        stop=(kt2 == 1),
                    )
                osb = out_pool.tile([P, 512], F32, tag="osb")
                nc.vector.tensor_copy(osb[:], pso[:])
                nc.sync.dma_start(
                    out[tt * P : (tt + 1) * P, ob * 512 : (ob + 1) * 512], osb[:]
                )


_BUILD_CACHE = {}


def build():
    if "nc" in _BUILD_CACHE:
        return _BUILD_CACHE["nc"]
    nc = bacc.Bacc("TRN2", target_bir_lowering=False, debug=False)
    aps = {
        "xT": nc.dram_tensor("xT", [D, S], DT_PROJ, kind="ExternalInput").ap(),
        "wq": nc.dram_tensor("wq", [D, WCOLS], DT_PROJ, kind="ExternalInput").ap(),
        "wk": nc.dram_tensor("wk", [D, WCOLS], DT_PROJ, kind="ExternalInput").ap(),
        "wv": nc.dram_tensor("wv", [D, WCOLS], DT_PROJ, kind="ExternalInput").ap(),
        "wo": nc.dram_tensor("wo", [WCOLS, D], DT_PROJ, kind="ExternalInput").ap(),
        "tri": nc.dram_tensor("tri", [P, P], DT_ATTN, kind="ExternalInput").ap(),
        "out": nc.dram_tensor("out", [S, D], F32, kind="ExternalOutput").ap(),
    }
    with tile.TileContext(nc) as tc:
        _emit(tc, aps)
    nc.compile()
    _BUILD_CACHE["nc"] = nc
    return nc


def make_tri() -> np.ndarray:
    """tri[dj, t] = 1 if dj <= t else 0 (causal keep within a 128 block)."""
    dj = np.arange(P)[:, None]
    t = np.arange(P)[None, :]
    return prep(np.where(dj <= t, 1.0, 0.0).astype(np.float32), DT_ATTN)


def make_in_maps(x, w_qkv, w_out):
    tri = make_tri()
    in_maps = []
    for c in range(NCORES):
        b, g = c // 4, c % 4
        cs = slice(g * WCOLS, (g + 1) * WCOLS)
        in_maps.append(
            {
                "xT": prep(x[b].T, DT_PROJ),
                "wq": prep(w_qkv[:, g * WCOLS : (g + 1) * WCOLS], DT_PROJ),
                "wk": prep(w_qkv[:, D + g * WCOLS : D + (g + 1) * WCOLS], DT_PROJ),
                "wv": prep(
                    w_qkv[:, 2 * D + g * WCOLS : 2 * D + (g + 1) * WCOLS], DT_PROJ
                ),
                "wo": prep(w_out[cs, :], DT_PROJ),
                "tri": tri,
            }
        )
    return in_maps


def kernel(x, w_qkv, w_out, _trace=False):
    nc = build()
    in_maps = make_in_maps(
        np.asarray(x, np.float32), np.asarray(w_qkv, np.float32),
        np.asarray(w_out, np.float32),
    )
    res = bass_utils.run_bass_kernel_spmd(
        nc, in_maps, core_ids=list(range(NCORES)), trace=_trace
    )
    outs = [res.results[c]["out"] for c in range(NCORES)]
    full = np.stack(
        [sum(outs[b * 4 : (b + 1) * 4][1:], outs[b * 4]) for b in range(B)], axis=0
    )
    if _trace:
        kernel.last_results = res
    return full.astype(np.float32)
